# revision 10
# baseline (speedup 1.0000x reference)
"""Dilated correlation kernel for Trainium2 (Bass/Tile), self-contained.

Problem: feat1, feat2 [8, 256, 64, 128] f32. L2-normalize along channels,
then out[b, k, y, x] = sum_c f1n[b,c,y,x] * f2n_padded[b,c,y+dy*d, x+dx*d]
for k = (d, dy, dx), d in (1,2,4), dy,dx in [-4,4]. Output [8, 243, 64, 128].

Sharding: pure data-parallel, one batch element per NeuronCore (8 cores).

Per-core algorithm:
  1. Normalize both features (square on ACT, channel-reduce via ones-matmul
     on PE, rsqrt on ACT, broadcast-scale+cast to bf16 on DVE).
  2. For each output row y and each of 17 distinct row-shifts r: a banded
     Gram matrix G[x, u] = sum_c f1n[c,y,x] * f2n[c, y+shift_r, u] over a
     160-wide padded u window (two 128-channel accumulating matmuls, bf16).
  3. Bands go PSUM -> SBUF (DVE) -> DRAM scratch (flat layout).
  4. Diagonal extraction: batched DMA reads DRAM band with stride-161 APs
     (per-partition diag is legal on the DRAM side): B[x, (r, s)] = G_r[x, x+s].
  5. PE-transpose B chunks -> BT[(r,s), x]; DVE copy into per-group slabs
     BTs[(r_local*36+s), (y, x)].
  6. Output: one DMA per (d, dy, y-block) writes 9 dx-planes at once with
     contiguous 512B x-rows. Out-of-range rows stay zero (output pre-zeroed).
"""
import numpy as np

import concourse.bass as bass
import concourse.tile as tile
from concourse import mybir
from concourse.bass_utils import run_bass_kernel_spmd
from concourse.masks import make_identity

# ---------------------------------------------------------------- constants
B, C, H, W = 8, 256, 64, 128
RADIUS = 4
DILATIONS = (1, 2, 4)
PAD = 16                      # radius * max(dilation)
UW = W + 2 * PAD              # padded row width 160
K = len(DILATIONS) * (2 * RADIUS + 1) ** 2  # 243

SHIFTS = sorted({dy * d for d in DILATIONS for dy in range(-RADIUS, RADIUS + 1)})
NR = len(SHIFTS)              # 17 distinct row shifts
R_OF = {s: i for i, s in enumerate(SHIFTS)}

BW = 36                       # padded band width per shift (33 used)
GB = 3                        # bands per transpose group
NG = 6                        # ceil(17/3) transpose groups
YB = 16                       # y rows per output block
N_CORES = 8

_nc_cache = {}


def _build():
    nc = bass.Bass("TRN2")
    f1_d = nc.dram_tensor("feat1", [C, H, W], mybir.dt.float32, kind="ExternalInput")
    f2_d = nc.dram_tensor("feat2", [C, H, W], mybir.dt.float32, kind="ExternalInput")
    out_d = nc.dram_tensor("out", [K, H, W], mybir.dt.float32, kind="ExternalOutput")

    fp32 = mybir.dt.float32
    bf16 = mybir.dt.bfloat16

    with tile.TileContext(nc) as tc:
        with (
            tc.tile_pool(name="persist", bufs=1) as persist,
            tc.tile_pool(name="work", bufs=3) as work,
            tc.tile_pool(name="psum", bufs=2, space="PSUM") as psum_pool,
            tc.tile_pool(name="tpsum", bufs=2, space="PSUM") as tpsum_pool,
            tc.tile_pool(name="dram", bufs=4, space="DRAM") as dram_pool,
        ):
            # persistent slabs (per-partition bytes):
            # f1n: 2cb*64y*128x bf16 = 32KB ; f2n: 2cb*64y*160u bf16 = 40KB
            f1n = persist.tile([128, 2, H, W], bf16)
            f2n = persist.tile([128, 2, H, UW], bf16)
            ident = persist.tile([128, 128], fp32)
            make_identity(nc, ident)
            ones = persist.tile([128, 1], bf16)
            nc.vector.memset(ones, 1.0)
            nc.gpsimd.memset(f2n, 0.0)

            # BTs slabs: NG x [GB*36=108, YB*128] f32 = 8KB/p each
            bts = [persist.tile([GB * BW, YB * W], fp32, name=f"bts{g}") for g in range(NG)]

            # ---------------- normalization pre-pass ----------------
            rsrep = persist.tile([128, H * W], bf16)
            for (src, dst, xoff) in ((f1_d, f1n, 0), (f2_d, f2n, PAD)):
                for y in range(H):
                    raw = work.tile([128, 2, W], fp32, tag="raw")
                    nc.sync.dma_start(
                        out=raw,
                        in_=bass.AP(tensor=src, offset=y * W, ap=[[H * W, 128], [H * W * 128, 2], [1, W]]),
                    )
                    for cb in range(2):
                        nc.vector.tensor_copy(out=dst[:, cb, y, xoff : xoff + W], in_=raw[:, cb, :])
                    sq = work.tile([128, 2, W], bf16, tag="sq")
                    nc.scalar.activation(out=sq, in_=raw, func=mybir.ActivationFunctionType.Square, scale=1.0)
                    nrm = tpsum_pool.tile([1, W], fp32, tag="tp")
                    nc.tensor.matmul(nrm, ones, sq[:, 0, :], start=True, stop=False)
                    nc.tensor.matmul(nrm, ones, sq[:, 1, :], start=False, stop=True)
                    rs = work.tile([1, W], fp32, tag="rs")
                    nc.scalar.activation(out=rs, in_=nrm, func=mybir.ActivationFunctionType.Sqrt, scale=1.0)
                    nc.vector.reciprocal(out=rs, in_=rs)
                    nc.vector.tensor_copy(out=rsrep[0:1, y * W : (y + 1) * W], in_=rs)
                # replicate partition 0 across all partitions (log-doubling)
                k = 1
                while k < 128:
                    nc.sync.dma_start(
                        out=bass.AP(tensor=rsrep.tensor, offset=rsrep.offset + k * H * W,
                                    ap=[[H * W, k], [1, H * W]]),
                        in_=bass.AP(tensor=rsrep.tensor, offset=rsrep.offset,
                                    ap=[[H * W, k], [1, H * W]]),
                    )
                    k *= 2
                # scale in place
                for y in range(H):
                    for cb in range(2):
                        nc.vector.tensor_mul(
                            out=dst[:, cb, y, xoff : xoff + W],
                            in0=dst[:, cb, y, xoff : xoff + W],
                            in1=rsrep[:, y * W : (y + 1) * W],
                        )

            # ---------------- main loop ----------------
            # shift value for each group slot g*GB+j ; shifts beyond NR are pad
            for yb in range(H // YB):
                for yy in range(YB):
                    y = yb * YB + yy
                    # --- Gram bands for the 17 shifts, in 3 psum groups of 6
                    gs = work.tile([128, NR * UW], fp32, tag="gs")
                    for grp_start in range(0, NR, 3):
                        grp = list(range(grp_start, min(grp_start + 3, NR)))
                        # one PSUM bank (512 f32) per band slice — matmuls must
                        # not straddle bank boundaries
                        pg = psum_pool.tile([128, len(grp), 512], fp32, tag="pg")
                        for j, r in enumerate(grp):
                            y2 = min(max(y + SHIFTS[r], 0), H - 1)
                            for cb in range(2):
                                nc.tensor.matmul(
                                    pg[:, j, 0:UW],
                                    f1n[:, cb, y, :],
                                    f2n[:, cb, y2, :],
                                    start=(cb == 0),
                                    stop=(cb == 1),
                                )
                        nc.vector.tensor_copy(
                            out=gs[:, grp_start * UW : (grp_start + len(grp)) * UW].rearrange(
                                "p (g u) -> p g u", u=UW
                            ),
                            in_=pg[:, :, 0:UW],
                        )
                    # --- band to DRAM
                    bdram = dram_pool.tile([128, NR * UW], fp32, tag="bdram")
                    nc.sync.dma_start(out=bdram, in_=gs)
                    # --- diagonal read-back: Bt[x, (r, s)] = G_r[x, x+s]
                    bt = work.tile([128, NR * BW], fp32, tag="bt")
                    din = bass.AP(tensor=bdram.tensor, offset=bdram.offset,
                                  ap=[[NR * UW + 1, 128], [UW, NR], [1, 33]])
                    dout = bass.AP(tensor=bt.tensor, offset=bt.offset,
                                   ap=[[NR * BW, 128], [BW, NR], [1, 33]])
                    nc.sync.dma_start(out=dout, in_=din)
                    # --- transpose groups of GB bands and store into BTs slabs
                    for g in range(NG):
                        nb = min(GB, NR - g * GB) * BW  # 108 or 72 for last
                        tp = tpsum_pool.tile([GB * BW, 128], fp32, tag="tp")
                        nc.tensor.transpose(
                            tp[0:nb, :], bt[:, g * GB * BW : g * GB * BW + nb], ident
                        )
                        nc.vector.tensor_copy(
                            out=bts[g][0:nb, yy * W : (yy + 1) * W], in_=tp[0:nb, :]
                        )
                # --- output DMAs for this y-block: one per (d, dy)
                for di, d in enumerate(DILATIONS):
                    for dy in range(-RADIUS, RADIUS + 1):
                        r = R_OF[dy * d]
                        g, gl = divmod(r, GB)
                        # valid y range within this block
                        ylo = max(0, -dy * d)
                        yhi = min(H, H - dy * d)
                        blo = max(ylo, yb * YB)
                        bhi = min(yhi, (yb + 1) * YB)
                        if blo >= bhi:
                            continue
                        k0 = di * 81 + (dy + RADIUS) * 9
                        srow = gl * BW + PAD - RADIUS * d  # s = 16 + dx*d, dx=-4
                        src = bass.AP(
                            tensor=bts[g].tensor,
                            offset=bts[g].offset + srow * (YB * W) + (blo - yb * YB) * W,
                            ap=[[d * YB * W, 9], [W, bhi - blo], [1, W]],
                        )
                        dst = bass.AP(
                            tensor=out_d, offset=k0 * H * W + blo * W,
                            ap=[[H * W, 9], [W, bhi - blo], [1, W]],
                        )
                        nc.sync.dma_start(out=dst, in_=src)

    from tile_fix_inline import split_multiwaits
    split_multiwaits(nc)
    return nc


# Inline copy of the multiwait workaround so kernel.py is self-contained.
import sys as _sys
import types as _types

_tf = _types.ModuleType("tile_fix_inline")


def _split_multiwaits(nc):
    import bass_rust

    ctr = [0]
    for f in nc.m.functions:
        for bb in f.blocks:
            insts = bb.instructions
            out = []
            changed = False
            for inst in insts:
                si = inst.sync_info
                if si is not None and len(si.on_wait) > 1:
                    waits = list(si.on_wait)
                    for w in waits[:-1]:
                        ctr[0] += 1
                        nop = mybir.InstNoOp(
                            name=f"mwfix-{ctr[0]}", engine=inst.engine, ins=[], outs=[]
                        )
                        nop.sync_info = bass_rust.SyncInfo(on_wait=[w], on_update=[])
                        out.append(nop)
                    inst.sync_info = bass_rust.SyncInfo(
                        on_wait=[waits[-1]], on_update=si.on_update
                    )
                    changed = True
                out.append(inst)
            if changed:
                insts.clear()
                insts.extend(out)


_tf.split_multiwaits = _split_multiwaits
_sys.modules["tile_fix_inline"] = _tf


def kernel(feat1: np.ndarray, feat2: np.ndarray) -> np.ndarray:
    feat1 = np.ascontiguousarray(feat1, dtype=np.float32)
    feat2 = np.ascontiguousarray(feat2, dtype=np.float32)
    assert feat1.shape == (B, C, H, W) and feat2.shape == (B, C, H, W)

    if "nc" not in _nc_cache:
        _nc_cache["nc"] = _build()
    nc = _nc_cache["nc"]

    in_maps = [
        {"feat1": feat1[b], "feat2": feat2[b]} for b in range(B)
    ]
    res = run_bass_kernel_spmd(nc, in_maps, core_ids=list(range(N_CORES)))
    out = np.stack([res.results[b]["out"] for b in range(B)], axis=0)
    return out
